# revision 37
# baseline (speedup 1.0000x reference)
"""BondInfluenceSelfAttention TRN2 kernel (fp16 matmul datapath).

Full-input contract: kernel(**inputs) takes the complete unsharded inputs and
returns the full [B, L, D] output. Internally shards across 8 NeuronCores:
core c handles batch b = c // 4 and head-group g = c % 4 (4 heads, 256 dk
dims). Each core computes its heads' attention plus the partial output
projection through its 256 rows of Wo; the host sums the 4 partials per batch
and adds bo.

Device-side formulation (per core). All matmul operands are fp16; PSUM
accumulates fp32:
  KT = Wk_g^T x^T       [256, L]
  QT = (Wq_g/8)^T x^T   [256, L]   (1/sqrt(dk)=1/8 folded into Wq/bq on host)
  V  = x Wv_g           [L, 256]   (bias via an appended ones-row matmul; a
                                    ones column rides along so the softmax
                                    denominator accumulates in row 64)
  S^T tile = K Q^T      [L_k, L_q] per head (dk=64 contraction)
  P = exp(S^T * bondT)  bond multiply on DVE out of PSUM, exp on ACT over
                        2 L_k-tiles x 4 heads at once (N=4096), fp16 out.
  1/denom = exp(-ln d) on ACT; the reciprocal row is replicated across 64
  partitions by a step-0 free-dim SBUF->SBUF DMA, then DVE normalizes O^T
  out as fp16.
  Y = O Wo_g fp16 matmuls, fp32 out, summed across head-group cores on host.

Scheduling. Two hardware facts drive the structure: (1) concurrent
Scalar-engine (ACT) activity throttles the PE to ~50% utilization
(activity_1 DVFS limit), while DVE/DMA activity does not, so every
PSUM->SBUF copy that can use DVE does, and ACT runs only what needs its
tables (Exp/Ln); (2) the PE needs a gapless stream to hold its fast pstate,
so the in-order TensorE queue must never park behind DVE/ACT work:
  - preamble is only K+Q of chunk 0 (~1.7 MB of DMA), K of chunks 1-3
    projects just-in-time inside chunk 0's group loop, V per L-tile;
  - AV matmuls trail the exp stage by two groups and interleave with the
    score matmuls slice-by-slice;
  - each chunk's normalize runs at the top of the next chunk and its output
    projection inside the next chunk's group 1; the last chunk cascades in
    two column halves to shorten the exposed tail.
"""

import numpy as np

try:
    import concourse.bass as bass  # noqa: F401
except ImportError:  # pragma: no cover
    import sys

    sys.path.insert(0, "/opt/trn_rl_repo")
    import concourse.bass as bass  # noqa: F401

import concourse.bacc as bacc
import concourse.mybir as mybir
import concourse.tile as tile
from concourse.bass_utils import run_bass_kernel_spmd

F32 = mybir.dt.float32
F16 = mybir.dt.float16

# Force Exp and Ln to resolve to the one activation-table set that holds
# both ("natural_log_exp_and_others"). Without this, Exp lands in set 0 and
# Ln in set 5, and every chunk boundary pays two ~1.3us ACT_TABLE_LOADs plus
# a serialized ACT pipeline that stalls the PE ~10us.
_ORIG_GET_ACT_TABLES = bacc.get_activation_tables


def _merged_act_tables(arch):
    tables = _ORIG_GET_ACT_TABLES(arch)
    both = "natural_log_exp_and_others"
    if both in tables:
        funcs = [
            mybir.ActivationFunctionType.Exp,
            mybir.ActivationFunctionType.Ln,
            mybir.ActivationFunctionType.Identity,
        ]
        if all(f in tables[both] for f in funcs):
            for name, fns in tables.items():
                if name != both:
                    for f in funcs:
                        fns.discard(f)
    return tables


bacc.get_activation_tables = _merged_act_tables

D = 1024  # d_model
L = 2048  # sequence length
B = 2  # batch
HPC = 4  # heads per core
DKG = 256  # dk dims per core (4 heads x 64)
NK = D // 128  # 8 contraction k-tiles for the projections
LT = L // 128  # 16 L-tiles
NCH = L // 512  # 4 L_q chunks
GRP = 2  # L_k tiles per exp group
NG = LT // GRP  # 8 groups per chunk
N_CORES = 8

_CACHED_NC = None


def _build_nc():
    nc = bacc.Bacc("TRN2", target_bir_lowering=False, debug=False, num_devices=N_CORES)

    xt_d = nc.declare_dram_parameter("xt", [D, L], F16, isOutput=False)
    bd_d = nc.declare_dram_parameter("bd", [L, L], F16, isOutput=False)
    wq_d = nc.declare_dram_parameter("wq", [D, DKG], F16, isOutput=False)
    wk_d = nc.declare_dram_parameter("wk", [D, DKG], F16, isOutput=False)
    wv_d = nc.declare_dram_parameter("wv", [D, DKG], F16, isOutput=False)
    bqk_d = nc.declare_dram_parameter("bqk", [128, 4], F32, isOutput=False)
    bv_d = nc.declare_dram_parameter("bv", [1, DKG], F16, isOutput=False)
    wo_d = nc.declare_dram_parameter("wo", [DKG, D], F16, isOutput=False)
    y_d = nc.declare_dram_parameter("y", [L, D], F16, isOutput=True)

    Exp = mybir.ActivationFunctionType.Exp
    Ln = mybir.ActivationFunctionType.Ln
    Identity = mybir.ActivationFunctionType.Identity

    with tile.TileContext(nc) as tc:
        with tc.tile_pool(name="persist", bufs=1) as pp:
            qt = [
                [
                    pp.tile([128, 512], F16, tag=f"qt{c}_{t}", name=f"qt{c}_{t}")
                    for t in range(2)
                ]
                for c in range(NCH)
            ]
            kt = [
                [
                    pp.tile([128, 512], F16, tag=f"kt{c}_{t}", name=f"kt{c}_{t}")
                    for t in range(2)
                ]
                for c in range(NCH)
            ]
            # cols 0:64 hold V, cols 64:128 hold ones so the AV matmul
            # replicates the softmax denominator across PSUM rows 64:128
            # (M=128 streams at the same N-bound rate as M=65)
            vt = [
                pp.tile([128, HPC, 128], F16, tag=f"v{i}", name=f"v{i}")
                for i in range(LT)
            ]
            ot = [pp.tile([128, L], F16, tag=f"ot{t}", name=f"ot{t}") for t in range(2)]
            wo_sb = pp.tile([128, 2, D], F16, tag="wo", name="wo_sb")
            bqk_sb = pp.tile([128, 4], F32, tag="bqk", name="bqk_sb")
            bv_sb = pp.tile([1, DKG], F16, tag="bv", name="bv_sb")
            ones_f = pp.tile([128, 128], F32, tag="onesf", name="ones_f")
            onesv = pp.tile([1, 128], F16, tag="onesv", name="onesv")
            ones16 = pp.tile([128, 64], F16, tag="ones16", name="ones16")

            nc.vector.memset(ones_f, 1.0)
            nc.vector.tensor_copy(out=onesv, in_=ones_f[0:1, :])
            nc.vector.tensor_copy(out=ones16, in_=ones_f[:, 0:64])
            for i in range(LT):
                nc.vector.memset(vt[i][:, :, 64:128], 1.0)

            with tc.tile_pool(name="xw", bufs=1) as xw, tc.tile_pool(
                name="ps", bufs=1, space="PSUM"
            ) as ps, tc.tile_pool(name="att", bufs=1) as att:
                xkc = [
                    [
                        xw.tile([128, 512], F16, tag=f"x{c}_{k}", name=f"x{c}_{k}")
                        for k in range(NK)
                    ]
                    for c in range(NCH)
                ]
                wq_k = [
                    xw.tile([128, DKG], F16, tag=f"wqk{k}", name=f"wqk{k}")
                    for k in range(NK)
                ]
                wk_k = [
                    xw.tile([128, DKG], F16, tag=f"wkk{k}", name=f"wkk{k}")
                    for k in range(NK)
                ]
                wv_sb = xw.tile([128, NK, DKG], F16, tag="wv", name="wv_sb")

                # input DMAs in consumption order across 3 queues, k-sliced
                # per-tile so the first K-proj burst starts after ~200 KB
                bd_g = bd_d.ap().rearrange("(g t p) l -> g p t l", p=128, t=GRP)
                pre_bt = {
                    g: att.tile([128, GRP, 512], F16, tag="bond", bufs=3, name="bt_pre")
                    for g in range(2)
                }
                xt_t = xt_d.ap().rearrange("(k p) (c l) -> k c p l", p=128, l=512)
                wk_t = wk_d.ap().rearrange("(k p) n -> k p n", p=128)
                wq_t = wq_d.ap().rearrange("(k p) n -> k p n", p=128)
                dmas = [(bqk_sb, bqk_d[:, :])]
                for k in range(NK):
                    dmas.append((wk_k[k][:, :], wk_t[k]))
                    dmas.append((xkc[0][k][:, :], xt_t[k][0]))
                for k in range(NK):
                    dmas.append((wq_k[k][:, :], wq_t[k]))
                # chunk-0 bond tiles right after wq so attention group 0
                # doesn't park behind the x1-x3/wo preamble traffic
                for g in range(2):
                    dmas.append((pre_bt[g], bd_g[g][:, :, 0:512]))
                dmas.append((wv_sb, wv_d.ap().rearrange("(k p) n -> p k n", p=128)))
                dmas.append((bv_sb[:, :], bv_d[:, :]))
                for k in range(NK):
                    dmas.append((xkc[1][k][:, :], xt_t[k][1]))
                for k in range(NK):
                    dmas.append((xkc[2][k][:, :], xt_t[k][2]))
                # (x columns for chunks 2-3 and wo follow below)
                for k in range(NK):
                    dmas.append((xkc[3][k][:, :], xt_t[k][3]))
                dmas.append((wo_sb, wo_d.ap().rearrange("(t p) n -> p t n", p=128)))
                queues = [nc.sync, nc.gpsimd, nc.scalar]
                for qi, (dst, src) in enumerate(dmas):
                    queues[qi % 3].dma_start(out=dst, in_=src)

                # warm the PE (HAM un-throttle needs ~3.4us of activity) while
                # the first K/Q-projection DMAs are still in flight
                warm = ps.tile([128, 512], F32, tag="oacc", bufs=4, name="warm")
                for _ in range(36):
                    nc.tensor.matmul(
                        warm[0:64, 0:64], ones16, ones16, start=True, stop=True
                    )

                def kproj(t, c, tag):
                    pk = ps.tile([128, 512], F32, tag=tag, bufs=4 if tag == "oacc" else 2, name="pk")
                    for k in range(NK):
                        nc.tensor.matmul(
                            pk[:, :],
                            wk_k[k][:, 128 * t : 128 * (t + 1)],
                            xkc[c][k][:, :],
                            start=(k == 0),
                            stop=(k == NK - 1),
                        )
                    nc.scalar.activation(
                        out=kt[c][t][:, :],
                        in_=pk[:, :],
                        func=Identity,
                        bias=bqk_sb[:, 2 + t : 3 + t],
                    )

                def qproj(t, c, tag):
                    pq = ps.tile([128, 512], F32, tag=tag, bufs=4 if tag == "oacc" else 2, name="pq")
                    for k in range(NK):
                        nc.tensor.matmul(
                            pq[:, :],
                            wq_k[k][:, 128 * t : 128 * (t + 1)],
                            xkc[c][k][:, :],
                            start=(k == 0),
                            stop=(k == NK - 1),
                        )
                    nc.scalar.activation(
                        out=qt[c][t][:, :],
                        in_=pq[:, :],
                        func=Identity,
                        bias=bqk_sb[:, t : t + 1],
                    )

                def vproj(i):
                    pv = ps.tile([128, DKG], F32, tag="s", bufs=2, name="pv")
                    for k in range(NK):
                        nc.tensor.matmul(
                            pv[:, :],
                            xkc[i // 4][k][:, 128 * (i % 4) : 128 * (i % 4 + 1)],
                            wv_sb[:, k, :],
                            start=(k == 0),
                            stop=False,
                        )
                    nc.tensor.matmul(
                        pv[:, :], onesv[:, :], bv_sb[:, :], start=False, stop=True
                    )
                    nc.scalar.activation(
                        out=vt[i][:, :, 0:64],
                        in_=pv.rearrange("p (h e) -> p h e", e=64),
                        func=Identity,
                    )

                # ---------------- preamble: K + Q of chunk 0, first V tiles -
                for t in range(2):
                    kproj(t, 0, "oacc")
                for t in range(2):
                    qproj(t, 0, "oacc")
                for i in range(6):
                    vproj(i)

                # ---------------- attention chunks --------------------------
                pend_yproj = None  # (yp tiles of prev chunk, chunk index)
                pend_norm = None  # (reciprocal tile, oaccs, chunk index)

                yq = [nc.gpsimd, nc.sync, nc.scalar]

                def issue_yproj(pend):
                    yps, cprev, lo = pend
                    pairs = [
                        (j, dh)
                        for j in range(4 * cprev, 4 * cprev + 4)
                        for dh in range(2)
                    ][lo : lo + 4]
                    for idx, (j, dh) in enumerate(pairs):
                        yp = yps[idx]
                        for t in range(2):
                            nc.tensor.matmul(
                                yp[:, :],
                                ot[t][:, 128 * j : 128 * (j + 1)],
                                wo_sb[:, t, 512 * dh : 512 * (dh + 1)],
                                start=(t == 0),
                                stop=(t == 1),
                            )
                        ys = att.tile([128, 512], F16, tag="ys", bufs=4, name="ys")
                        if dh == 0:
                            nc.scalar.activation(out=ys, in_=yp[:, :], func=Identity)
                        else:
                            nc.vector.tensor_copy(out=ys, in_=yp[:, :])
                        yq[idx % 2].dma_start(
                            out=y_d[128 * j : 128 * (j + 1), 512 * dh : 512 * (dh + 1)],
                            in_=ys,
                        )

                def issue_recip(oaccs_p):
                    # 1/denom = exp(-ln d) on ACT; the denominators sit
                    # replicated on partitions 64:128 of each accumulator
                    # (ones block in vt), so the ACT output is already 64
                    # rows wide; one DMA shifts it to partitions 0:64 where
                    # the normalize multiply needs it.
                    ld = att.tile([128, HPC, 512], F32, tag="ld", bufs=1, name="ld")
                    rd = att.tile([128, HPC, 512], F16, tag="rd", bufs=1, name="rd")
                    bcs = att.tile([64, HPC, 512], F16, tag="bcs", bufs=2, name="bcs")
                    for h in range(HPC):
                        nc.scalar.activation(
                            out=ld[64:128, h, :], in_=oaccs_p[h][64:128, :], func=Ln
                        )
                    nc.scalar.activation(
                        out=rd[64:128, :, :], in_=ld[64:128, :, :], func=Exp, scale=-1.0
                    )
                    nc.gpsimd.dma_start(out=bcs, in_=rd[64:128, :, :])
                    return bcs

                def issue_norm(bcs_t, oaccs_p, cprev):
                    for t in range(2):
                        for half in range(2):
                            h = 2 * t + half
                            bcs = bcs_t[:, h, :]
                            if half == 0:
                                nc.vector.tensor_mul(
                                    out=ot[t][0:64, 512 * cprev : 512 * (cprev + 1)],
                                    in0=oaccs_p[h][0:64, :],
                                    in1=bcs,
                                )
                            else:
                                odd = att.tile([64, 512], F16, tag="odd", bufs=2, name="odd")
                                nc.vector.tensor_mul(
                                    out=odd,
                                    in0=oaccs_p[h][0:64, :],
                                    in1=bcs,
                                )
                                nc.gpsimd.dma_start(
                                    out=ot[t][64:128, 512 * cprev : 512 * (cprev + 1)],
                                    in_=odd,
                                )

                # chunk-0 just-in-time projections: kt[] is indexed by KEY
                # chunk, so chunk 0's own score sweep needs all four kt
                # chunks (kt[n] by group 2n); Q is per-query-chunk
                c0_jobs = {
                    (0, 0): ("k", 0, 1),
                    (0, 1): ("k", 1, 1),
                    (1, 0): ("k", 0, 2),
                    (1, 1): ("k", 1, 2),
                    (2, 0): ("k", 0, 3),
                    (2, 1): ("k", 1, 3),
                    (3, 0): ("q", 0, 1),
                    (3, 1): ("q", 1, 1),
                }
                # steady chunks host only chunk c+1's Q, at groups 1/6
                # (clear of the boundary-hook window g2-g5)
                jit_q = {(1, 0): 0, (6, 0): 1}

                # -------- flat pipeline over all (c, g) groups --------------
                # The score/TT/exp front-end streams continuously across
                # chunk boundaries; the AV stage trails by >=3 groups and
                # stretches over a boundary until the previous chunk's
                # recip/norm/yproj have freed the PSUM accumulators.
                oaccs_by_c = {}
                pend = []  # (pt tile, chunk, group) awaiting AV
                hooks = []  # deferred per-chunk work, one per pipeline step

                def alloc_oaccs(cc):
                    oaccs_by_c[cc] = [
                        ps.tile([128, 512], F32, tag="oacc", bufs=4, name=f"oacc{h}")
                        for h in range(HPC)
                    ]

                def av_slice(pt_g, cc, gc, ii):
                    i = GRP * gc + ii
                    oaccs = oaccs_by_c[cc]
                    for h in range(HPC):
                        nc.tensor.matmul(
                            oaccs[h][:, :],
                            vt[i][:, h, :],
                            pt_g[:, ii, h, :],
                            start=(i == 0),
                            stop=(i == LT - 1),
                        )

                def av_ready():
                    return pend and pend[0][1] in oaccs_by_c

                def av_block():
                    pt_g, cc, gc = pend.pop(0)
                    for ii in range(GRP):
                        av_slice(pt_g, cc, gc, ii)
                    if gc == NG - 1:
                        finish_chunk(cc)

                def finish_chunk(cc):
                    # last AV of chunk cc just issued: reciprocal now, then
                    # normalize / output projection on following steps
                    oaccs = oaccs_by_c.pop(cc)
                    bcs = issue_recip(oaccs)
                    hooks.append(lambda: issue_norm(bcs, oaccs, cc))

                    def yproj_half(lo):
                        def run():
                            yps = [
                                ps.tile([128, 512], F32, tag="oacc", bufs=4, name="yp")
                                for _ in range(4)
                            ]
                            issue_yproj((yps, cc, lo))
                            if lo == 4 and cc + 1 < NCH:
                                alloc_oaccs(cc + 1)

                        return run

                    hooks.append(yproj_half(0))
                    hooks.append(yproj_half(4))

                alloc_oaccs(0)
                for c in range(NCH):
                    for g in range(NG):
                        av_cur = (
                            pend.pop(0)
                            if len(pend) >= 3 and pend[0][1] in oaccs_by_c
                            else None
                        )
                        prod = att.tile(
                            [128, GRP, HPC, 512], F16, tag="prod", bufs=3, name="prod"
                        )
                        pt_g = att.tile(
                            [128, GRP, HPC, 512], F16, tag="pt", bufs=7, name="pt"
                        )
                        if c == 0 and g in pre_bt:
                            bt2 = pre_bt[g]
                        else:
                            bt2 = att.tile(
                                [128, GRP, 512], F16, tag="bond", bufs=3, name="bt2"
                            )
                            nc.sync.dma_start(
                                out=bt2,
                                in_=bd_g[g][:, :, 512 * c : 512 * (c + 1)],
                            )
                        for ii in range(GRP):
                            i = GRP * g + ii
                            if c == 0 and g >= 3:
                                vproj(6 + GRP * (g - 3) + ii)
                            if c == 0 and (g, ii) in c0_jobs:
                                kind, tj, cj = c0_jobs[(g, ii)]
                                if kind == "k":
                                    kproj(tj, cj, "s")
                                else:
                                    qproj(tj, cj, "s")
                            elif c < NCH - 1 and (g, ii) in jit_q:
                                qproj(jit_q[(g, ii)], c + 1, "s")
                            bt = bt2[:, ii, :]
                            bt_bcast = bass.AP(
                                tensor=bt.tensor,
                                offset=bt.offset,
                                ap=[bt.ap[0], [0, 2]] + list(bt.ap[1:]),
                            )
                            for t in range(2):
                                spair = ps.tile(
                                    [128, 2, 512], F32, tag="s", bufs=2, name="spair"
                                )
                                for half in range(2):
                                    nc.tensor.matmul(
                                        spair[:, half, :],
                                        kt[i // 4][t][64 * half : 64 * (half + 1), 128 * (i % 4) : 128 * (i % 4 + 1)],
                                        qt[c][t][64 * half : 64 * (half + 1), :],
                                        start=True,
                                        stop=True,
                                    )
                                nc.vector.tensor_mul(
                                    out=prod[:, ii, 2 * t : 2 * (t + 1), :],
                                    in0=spair,
                                    in1=bt_bcast,
                                )
                            if av_cur is not None:
                                av_slice(av_cur[0], av_cur[1], av_cur[2], ii)
                        nc.scalar.activation(out=pt_g, in_=prod, func=Exp)
                        if av_cur is not None and av_cur[2] == NG - 1:
                            finish_chunk(av_cur[1])
                        pend.append((pt_g, c, g))
                        if hooks:
                            hooks.pop(0)()
                        # catch up the AV backlog accumulated over a boundary
                        while len(pend) > 3 and av_ready():
                            av_block()

                # drain: remaining AV groups, then the exposed tail chain
                while pend or hooks:
                    if hooks:
                        hooks.pop(0)()
                    if av_ready():
                        av_block()

    nc.compile()
    return nc


def _get_nc():
    global _CACHED_NC
    if _CACHED_NC is None:
        _CACHED_NC = _build_nc()
    return _CACHED_NC


def _host_prep(x, bond_influence, Wq, bq, Wk, bk, Wv, bv, Wo):
    xt_b = [np.ascontiguousarray(x[b].T.astype(np.float16)) for b in range(B)]
    bd_b = [
        np.ascontiguousarray(bond_influence[b].T.astype(np.float16)) for b in range(B)
    ]
    in_maps = []
    for core in range(N_CORES):
        b, g = core // HPC, core % HPC
        s = slice(g * DKG, (g + 1) * DKG)
        bq_g = (bq[s] / 8.0).astype(np.float32)
        bk_g = bk[s].astype(np.float32)
        bqk = np.stack(
            [bq_g[0:128], bq_g[128:256], bk_g[0:128], bk_g[128:256]], axis=1
        )
        in_maps.append(
            {
                "xt": xt_b[b],
                "bd": bd_b[b],
                "wq": np.ascontiguousarray((Wq[:, s] / 8.0).astype(np.float16)),
                "wk": np.ascontiguousarray(Wk[:, s].astype(np.float16)),
                "wv": np.ascontiguousarray(Wv[:, s].astype(np.float16)),
                "bqk": np.ascontiguousarray(bqk),
                "bv": np.ascontiguousarray(bv[s][None, :].astype(np.float16)),
                "wo": np.ascontiguousarray(Wo[s, :].astype(np.float16)),
            }
        )
    return in_maps


def kernel(
    x,
    bond_influence,
    Wq,
    bq,
    Wk,
    bk,
    Wv,
    bv,
    Wo,
    bo,
    _trace=False,
    _trace_out=None,
):
    x = np.asarray(x, dtype=np.float32)
    bond_influence = np.asarray(bond_influence, dtype=np.float32)
    args = [np.asarray(a, dtype=np.float32) for a in (Wq, bq, Wk, bk, Wv, bv, Wo)]
    bo = np.asarray(bo, dtype=np.float32)

    nc = _get_nc()
    in_maps = _host_prep(x, bond_influence, *args)
    kwargs = {}
    if _trace:
        kwargs = dict(trace=True, tmpdir=_trace_out)
    res = run_bass_kernel_spmd(nc, in_maps, list(range(N_CORES)), **kwargs)

    out = np.zeros((B, L, D), dtype=np.float32)
    for b in range(B):
        acc = res.results[4 * b]["y"].astype(np.float32).copy()
        for g in range(1, HPC):
            acc += res.results[4 * b + g]["y"]
        out[b] = acc + bo[None, :]
    if _trace:
        return out, res
    return out



# revision 38
# speedup vs baseline: 1.0367x; 1.0367x over previous
"""BondInfluenceSelfAttention TRN2 kernel (fp16 matmul datapath).

Full-input contract: kernel(**inputs) takes the complete unsharded inputs and
returns the full [B, L, D] output. Internally shards across 8 NeuronCores:
core c handles batch b = c // 4 and head-group g = c % 4 (4 heads, 256 dk
dims). Each core computes its heads' attention plus the partial output
projection through its 256 rows of Wo; the host sums the 4 partials per batch
and adds bo.

Device-side formulation (per core). All matmul operands are fp16; PSUM
accumulates fp32:
  KT = Wk_g^T x^T       [256, L]
  QT = (Wq_g/8)^T x^T   [256, L]   (1/sqrt(dk)=1/8 folded into Wq/bq on host)
  V  = x Wv_g           [L, 256]   (bias via an appended ones-row matmul; a
                                    ones column rides along so the softmax
                                    denominator accumulates in row 64)
  S^T tile = K Q^T      [L_k, L_q] per head (dk=64 contraction)
  P = exp(S^T * bondT)  bond multiply on DVE out of PSUM, exp on ACT over
                        2 L_k-tiles x 4 heads at once (N=4096), fp16 out.
  1/denom = exp(-ln d) on ACT; the reciprocal row is replicated across 64
  partitions by a step-0 free-dim SBUF->SBUF DMA, then DVE normalizes O^T
  out as fp16.
  Y = O Wo_g fp16 matmuls, fp32 out, summed across head-group cores on host.

Scheduling. Two hardware facts drive the structure: (1) concurrent
Scalar-engine (ACT) activity throttles the PE to ~50% utilization
(activity_1 DVFS limit), while DVE/DMA activity does not, so every
PSUM->SBUF copy that can use DVE does, and ACT runs only what needs its
tables (Exp/Ln); (2) the PE needs a gapless stream to hold its fast pstate,
so the in-order TensorE queue must never park behind DVE/ACT work:
  - preamble is only K+Q of chunk 0 (~1.7 MB of DMA), K of chunks 1-3
    projects just-in-time inside chunk 0's group loop, V per L-tile;
  - AV matmuls trail the exp stage by two groups and interleave with the
    score matmuls slice-by-slice;
  - each chunk's normalize runs at the top of the next chunk and its output
    projection inside the next chunk's group 1; the last chunk cascades in
    two column halves to shorten the exposed tail.
"""

import numpy as np

try:
    import concourse.bass as bass  # noqa: F401
except ImportError:  # pragma: no cover
    import sys

    sys.path.insert(0, "/opt/trn_rl_repo")
    import concourse.bass as bass  # noqa: F401

import concourse.bacc as bacc
import concourse.mybir as mybir
import concourse.tile as tile
from concourse.bass_utils import run_bass_kernel_spmd

F32 = mybir.dt.float32
F16 = mybir.dt.float16

# Force Exp and Ln to resolve to the one activation-table set that holds
# both ("natural_log_exp_and_others"). Without this, Exp lands in set 0 and
# Ln in set 5, and every chunk boundary pays two ~1.3us ACT_TABLE_LOADs plus
# a serialized ACT pipeline that stalls the PE ~10us.
_ORIG_GET_ACT_TABLES = bacc.get_activation_tables


def _merged_act_tables(arch):
    tables = _ORIG_GET_ACT_TABLES(arch)
    both = "natural_log_exp_and_others"
    if both in tables:
        funcs = [
            mybir.ActivationFunctionType.Exp,
            mybir.ActivationFunctionType.Ln,
            mybir.ActivationFunctionType.Identity,
        ]
        if all(f in tables[both] for f in funcs):
            for name, fns in tables.items():
                if name != both:
                    for f in funcs:
                        fns.discard(f)
    return tables


bacc.get_activation_tables = _merged_act_tables

D = 1024  # d_model
L = 2048  # sequence length
B = 2  # batch
HPC = 4  # heads per core
DKG = 256  # dk dims per core (4 heads x 64)
NK = D // 128  # 8 contraction k-tiles for the projections
LT = L // 128  # 16 L-tiles
NCH = L // 512  # 4 L_q chunks
GRP = 2  # L_k tiles per exp group
NG = LT // GRP  # 8 groups per chunk
N_CORES = 8

_CACHED_NC = None


def _build_nc():
    nc = bacc.Bacc("TRN2", target_bir_lowering=False, debug=False, num_devices=N_CORES)

    xt_d = nc.declare_dram_parameter("xt", [D, L], F16, isOutput=False)
    bd_d = nc.declare_dram_parameter("bd", [L, L], F16, isOutput=False)
    wq_d = nc.declare_dram_parameter("wq", [D, DKG], F16, isOutput=False)
    wk_d = nc.declare_dram_parameter("wk", [D, DKG], F16, isOutput=False)
    wv_d = nc.declare_dram_parameter("wv", [D, DKG], F16, isOutput=False)
    bqk_d = nc.declare_dram_parameter("bqk", [128, 4], F32, isOutput=False)
    bv_d = nc.declare_dram_parameter("bv", [1, DKG], F16, isOutput=False)
    wo_d = nc.declare_dram_parameter("wo", [DKG, D], F16, isOutput=False)
    y_d = nc.declare_dram_parameter("y", [L, D], F16, isOutput=True)

    Exp = mybir.ActivationFunctionType.Exp
    Ln = mybir.ActivationFunctionType.Ln
    Identity = mybir.ActivationFunctionType.Identity

    with tile.TileContext(nc) as tc:
        with tc.tile_pool(name="persist", bufs=1) as pp:
            qt = [
                [
                    pp.tile([128, 512], F16, tag=f"qt{c}_{t}", name=f"qt{c}_{t}")
                    for t in range(2)
                ]
                for c in range(NCH)
            ]
            kt = [
                [
                    pp.tile([128, 512], F16, tag=f"kt{c}_{t}", name=f"kt{c}_{t}")
                    for t in range(2)
                ]
                for c in range(NCH)
            ]
            # cols 0:64 hold V, cols 64:128 hold ones so the AV matmul
            # replicates the softmax denominator across PSUM rows 64:128
            # (M=128 streams at the same N-bound rate as M=65)
            vt = [
                pp.tile([128, HPC, 128], F16, tag=f"v{i}", name=f"v{i}")
                for i in range(LT)
            ]
            ot = [pp.tile([128, L], F16, tag=f"ot{t}", name=f"ot{t}") for t in range(2)]
            wo_sb = pp.tile([128, 2, D], F16, tag="wo", name="wo_sb")
            bqk_sb = pp.tile([128, 4], F32, tag="bqk", name="bqk_sb")
            bv_sb = pp.tile([1, DKG], F16, tag="bv", name="bv_sb")
            ones_f = pp.tile([128, 128], F32, tag="onesf", name="ones_f")
            onesv = pp.tile([1, 128], F16, tag="onesv", name="onesv")
            ones16 = pp.tile([128, 64], F16, tag="ones16", name="ones16")

            nc.vector.memset(ones_f, 1.0)
            nc.vector.tensor_copy(out=onesv, in_=ones_f[0:1, :])
            nc.vector.tensor_copy(out=ones16, in_=ones_f[:, 0:64])
            for i in range(LT):
                nc.vector.memset(vt[i][:, :, 64:128], 1.0)

            with tc.tile_pool(name="xw", bufs=1) as xw, tc.tile_pool(
                name="ps", bufs=1, space="PSUM"
            ) as ps, tc.tile_pool(name="att", bufs=1) as att:
                xkc = [
                    [
                        xw.tile([128, 512], F16, tag=f"x{c}_{k}", name=f"x{c}_{k}")
                        for k in range(NK)
                    ]
                    for c in range(NCH)
                ]
                wq_k = [
                    xw.tile([128, DKG], F16, tag=f"wqk{k}", name=f"wqk{k}")
                    for k in range(NK)
                ]
                wk_k = [
                    xw.tile([128, DKG], F16, tag=f"wkk{k}", name=f"wkk{k}")
                    for k in range(NK)
                ]
                wv_sb = xw.tile([128, NK, DKG], F16, tag="wv", name="wv_sb")

                # input DMAs in consumption order across 3 queues, k-sliced
                # per-tile so the first K-proj burst starts after ~200 KB
                bd_g = bd_d.ap().rearrange("(g t p) l -> g p t l", p=128, t=GRP)
                pre_bt = {
                    g: att.tile([128, GRP, 512], F16, tag="bond", bufs=3, name="bt_pre")
                    for g in range(2)
                }
                xt_t = xt_d.ap().rearrange("(k p) (c l) -> k c p l", p=128, l=512)
                wk_t = wk_d.ap().rearrange("(k p) n -> k p n", p=128)
                wq_t = wq_d.ap().rearrange("(k p) n -> k p n", p=128)
                dmas = [(bqk_sb, bqk_d[:, :])]
                for k in range(NK):
                    dmas.append((wk_k[k][:, :], wk_t[k]))
                    dmas.append((xkc[0][k][:, :], xt_t[k][0]))
                for k in range(NK):
                    dmas.append((wq_k[k][:, :], wq_t[k]))
                # chunk-0 bond tiles right after wq so attention group 0
                # doesn't park behind the x1-x3/wo preamble traffic
                for g in range(2):
                    dmas.append((pre_bt[g], bd_g[g][:, :, 0:512]))
                dmas.append((wv_sb, wv_d.ap().rearrange("(k p) n -> p k n", p=128)))
                dmas.append((bv_sb[:, :], bv_d[:, :]))
                for k in range(NK):
                    dmas.append((xkc[1][k][:, :], xt_t[k][1]))
                for k in range(NK):
                    dmas.append((xkc[2][k][:, :], xt_t[k][2]))
                # (x columns for chunks 2-3 and wo follow below)
                for k in range(NK):
                    dmas.append((xkc[3][k][:, :], xt_t[k][3]))
                dmas.append((wo_sb, wo_d.ap().rearrange("(t p) n -> p t n", p=128)))
                queues = [nc.sync, nc.gpsimd, nc.scalar]
                for qi, (dst, src) in enumerate(dmas):
                    queues[qi % 3].dma_start(out=dst, in_=src)

                # warm the PE (HAM un-throttle needs ~3.4us of activity) while
                # the first K/Q-projection DMAs are still in flight
                warm = ps.tile([128, 512], F32, tag="oacc", bufs=4, name="warm")
                for _ in range(36):
                    nc.tensor.matmul(
                        warm[0:64, 0:64], ones16, ones16, start=True, stop=True
                    )

                def kproj(t, c, tag):
                    pk = ps.tile([128, 512], F32, tag=tag, bufs=4 if tag == "oacc" else 2, name="pk")
                    for k in range(NK):
                        nc.tensor.matmul(
                            pk[:, :],
                            wk_k[k][:, 128 * t : 128 * (t + 1)],
                            xkc[c][k][:, :],
                            start=(k == 0),
                            stop=(k == NK - 1),
                        )
                    nc.vector.tensor_scalar_add(
                        out=kt[c][t][:, :], in0=pk[:, :], scalar1=bqk_sb[:, 2 + t : 3 + t]
                    )

                def qproj(t, c, tag):
                    pq = ps.tile([128, 512], F32, tag=tag, bufs=4 if tag == "oacc" else 2, name="pq")
                    for k in range(NK):
                        nc.tensor.matmul(
                            pq[:, :],
                            wq_k[k][:, 128 * t : 128 * (t + 1)],
                            xkc[c][k][:, :],
                            start=(k == 0),
                            stop=(k == NK - 1),
                        )
                    nc.vector.tensor_scalar_add(
                        out=qt[c][t][:, :], in0=pq[:, :], scalar1=bqk_sb[:, t : t + 1]
                    )

                def vproj(i):
                    pv = ps.tile([128, DKG], F32, tag="s", bufs=2, name="pv")
                    for k in range(NK):
                        nc.tensor.matmul(
                            pv[:, :],
                            xkc[i // 4][k][:, 128 * (i % 4) : 128 * (i % 4 + 1)],
                            wv_sb[:, k, :],
                            start=(k == 0),
                            stop=False,
                        )
                    nc.tensor.matmul(
                        pv[:, :], onesv[:, :], bv_sb[:, :], start=False, stop=True
                    )
                    nc.vector.tensor_copy(
                        out=vt[i][:, :, 0:64],
                        in_=pv.rearrange("p (h e) -> p h e", e=64),
                    )

                # ---------------- preamble: K + Q of chunk 0, first V tiles -
                for t in range(2):
                    kproj(t, 0, "oacc")
                for t in range(2):
                    qproj(t, 0, "oacc")
                for i in range(6):
                    vproj(i)

                # ---------------- attention chunks --------------------------
                pend_yproj = None  # (yp tiles of prev chunk, chunk index)
                pend_norm = None  # (reciprocal tile, oaccs, chunk index)

                yq = [nc.gpsimd, nc.sync, nc.scalar]

                def issue_yproj(pend):
                    yps, cprev, lo = pend
                    pairs = [
                        (j, dh)
                        for j in range(4 * cprev, 4 * cprev + 4)
                        for dh in range(2)
                    ][lo : lo + 4]
                    for idx, (j, dh) in enumerate(pairs):
                        yp = yps[idx]
                        for t in range(2):
                            nc.tensor.matmul(
                                yp[:, :],
                                ot[t][:, 128 * j : 128 * (j + 1)],
                                wo_sb[:, t, 512 * dh : 512 * (dh + 1)],
                                start=(t == 0),
                                stop=(t == 1),
                            )
                        ys = att.tile([128, 512], F16, tag="ys", bufs=4, name="ys")
                        if dh == 0:
                            nc.scalar.activation(out=ys, in_=yp[:, :], func=Identity)
                        else:
                            nc.vector.tensor_copy(out=ys, in_=yp[:, :])
                        yq[idx % 2].dma_start(
                            out=y_d[128 * j : 128 * (j + 1), 512 * dh : 512 * (dh + 1)],
                            in_=ys,
                        )

                def issue_recip(oaccs_p):
                    # 1/denom = exp(-ln d) on ACT; the denominators sit
                    # replicated on partitions 64:128 of each accumulator
                    # (ones block in vt), so the ACT output is already 64
                    # rows wide; one DMA shifts it to partitions 0:64 where
                    # the normalize multiply needs it.
                    ld = att.tile([128, HPC, 512], F32, tag="ld", bufs=1, name="ld")
                    rd = att.tile([128, HPC, 512], F16, tag="rd", bufs=1, name="rd")
                    bcs = att.tile([64, HPC, 512], F16, tag="bcs", bufs=2, name="bcs")
                    for h in range(HPC):
                        nc.scalar.activation(
                            out=ld[64:128, h, :], in_=oaccs_p[h][64:128, :], func=Ln
                        )
                    nc.scalar.activation(
                        out=rd[64:128, :, :], in_=ld[64:128, :, :], func=Exp, scale=-1.0
                    )
                    nc.gpsimd.dma_start(out=bcs, in_=rd[64:128, :, :])
                    return bcs

                def issue_norm(bcs_t, oaccs_p, cprev):
                    for t in range(2):
                        for half in range(2):
                            h = 2 * t + half
                            bcs = bcs_t[:, h, :]
                            if half == 0:
                                nc.vector.tensor_mul(
                                    out=ot[t][0:64, 512 * cprev : 512 * (cprev + 1)],
                                    in0=oaccs_p[h][0:64, :],
                                    in1=bcs,
                                )
                            else:
                                odd = att.tile([64, 512], F16, tag="odd", bufs=2, name="odd")
                                nc.vector.tensor_mul(
                                    out=odd,
                                    in0=oaccs_p[h][0:64, :],
                                    in1=bcs,
                                )
                                nc.gpsimd.dma_start(
                                    out=ot[t][64:128, 512 * cprev : 512 * (cprev + 1)],
                                    in_=odd,
                                )

                # chunk-0 just-in-time projections: kt[] is indexed by KEY
                # chunk, so chunk 0's own score sweep needs all four kt
                # chunks (kt[n] by group 2n); Q is per-query-chunk
                c0_jobs = {
                    (0, 0): ("k", 0, 1),
                    (0, 1): ("k", 1, 1),
                    (1, 0): ("k", 0, 2),
                    (1, 1): ("k", 1, 2),
                    (2, 0): ("k", 0, 3),
                    (2, 1): ("k", 1, 3),
                    (3, 0): ("q", 0, 1),
                    (3, 1): ("q", 1, 1),
                }
                # steady chunks host only chunk c+1's Q, at groups 1/6
                # (clear of the boundary-hook window g2-g5)
                jit_q = {(1, 0): 0, (6, 0): 1}

                # -------- flat pipeline over all (c, g) groups --------------
                # The score/TT/exp front-end streams continuously across
                # chunk boundaries; the AV stage trails by >=3 groups and
                # stretches over a boundary until the previous chunk's
                # recip/norm/yproj have freed the PSUM accumulators.
                oaccs_by_c = {}
                pend = []  # (pt tile, chunk, group) awaiting AV
                hooks = []  # deferred per-chunk work, one per pipeline step

                def alloc_oaccs(cc):
                    oaccs_by_c[cc] = [
                        ps.tile([128, 512], F32, tag="oacc", bufs=4, name=f"oacc{h}")
                        for h in range(HPC)
                    ]

                def av_slice(pt_g, cc, gc, ii):
                    i = GRP * gc + ii
                    oaccs = oaccs_by_c[cc]
                    for h in range(HPC):
                        nc.tensor.matmul(
                            oaccs[h][:, :],
                            vt[i][:, h, :],
                            pt_g[:, ii, h, :],
                            start=(i == 0),
                            stop=(i == LT - 1),
                        )

                def av_ready():
                    return pend and pend[0][1] in oaccs_by_c

                def av_block():
                    pt_g, cc, gc = pend.pop(0)
                    for ii in range(GRP):
                        av_slice(pt_g, cc, gc, ii)
                    if gc == NG - 1:
                        finish_chunk(cc)

                def finish_chunk(cc):
                    # last AV of chunk cc just issued: reciprocal now, then
                    # normalize / output projection on following steps
                    oaccs = oaccs_by_c.pop(cc)
                    bcs = issue_recip(oaccs)
                    hooks.append(lambda: issue_norm(bcs, oaccs, cc))

                    def yproj_half(lo):
                        def run():
                            yps = [
                                ps.tile([128, 512], F32, tag="oacc", bufs=4, name="yp")
                                for _ in range(4)
                            ]
                            issue_yproj((yps, cc, lo))
                            if lo == 4 and cc + 1 < NCH:
                                alloc_oaccs(cc + 1)

                        return run

                    hooks.append(yproj_half(0))
                    hooks.append(yproj_half(4))

                alloc_oaccs(0)
                for c in range(NCH):
                    for g in range(NG):
                        av_cur = (
                            pend.pop(0)
                            if len(pend) >= 3 and pend[0][1] in oaccs_by_c
                            else None
                        )
                        prod = att.tile(
                            [128, GRP, HPC, 512], F16, tag="prod", bufs=3, name="prod"
                        )
                        pt_g = att.tile(
                            [128, GRP, HPC, 512], F16, tag="pt", bufs=7, name="pt"
                        )
                        if c == 0 and g in pre_bt:
                            bt2 = pre_bt[g]
                        else:
                            bt2 = att.tile(
                                [128, GRP, 512], F16, tag="bond", bufs=3, name="bt2"
                            )
                            nc.sync.dma_start(
                                out=bt2,
                                in_=bd_g[g][:, :, 512 * c : 512 * (c + 1)],
                            )
                        for ii in range(GRP):
                            i = GRP * g + ii
                            if c == 0 and g >= 3:
                                vproj(6 + GRP * (g - 3) + ii)
                            if c == 0 and (g, ii) in c0_jobs:
                                kind, tj, cj = c0_jobs[(g, ii)]
                                if kind == "k":
                                    kproj(tj, cj, "s")
                                else:
                                    qproj(tj, cj, "s")
                            elif c < NCH - 1 and (g, ii) in jit_q:
                                qproj(jit_q[(g, ii)], c + 1, "s")
                            bt = bt2[:, ii, :]
                            bt_bcast = bass.AP(
                                tensor=bt.tensor,
                                offset=bt.offset,
                                ap=[bt.ap[0], [0, 2]] + list(bt.ap[1:]),
                            )
                            for t in range(2):
                                spair = ps.tile(
                                    [128, 2, 512], F32, tag="s", bufs=2, name="spair"
                                )
                                for half in range(2):
                                    nc.tensor.matmul(
                                        spair[:, half, :],
                                        kt[i // 4][t][64 * half : 64 * (half + 1), 128 * (i % 4) : 128 * (i % 4 + 1)],
                                        qt[c][t][64 * half : 64 * (half + 1), :],
                                        start=True,
                                        stop=True,
                                    )
                                nc.vector.tensor_mul(
                                    out=prod[:, ii, 2 * t : 2 * (t + 1), :],
                                    in0=spair,
                                    in1=bt_bcast,
                                )
                            if av_cur is not None:
                                av_slice(av_cur[0], av_cur[1], av_cur[2], ii)
                        nc.scalar.activation(out=pt_g, in_=prod, func=Exp)
                        if av_cur is not None and av_cur[2] == NG - 1:
                            finish_chunk(av_cur[1])
                        pend.append((pt_g, c, g))
                        if hooks:
                            hooks.pop(0)()
                        # catch up the AV backlog accumulated over a boundary
                        while len(pend) > 3 and av_ready():
                            av_block()

                # drain: remaining AV groups, then the exposed tail chain
                while pend or hooks:
                    if hooks:
                        hooks.pop(0)()
                    if av_ready():
                        av_block()

    nc.compile()
    return nc


def _get_nc():
    global _CACHED_NC
    if _CACHED_NC is None:
        _CACHED_NC = _build_nc()
    return _CACHED_NC


def _host_prep(x, bond_influence, Wq, bq, Wk, bk, Wv, bv, Wo):
    xt_b = [np.ascontiguousarray(x[b].T.astype(np.float16)) for b in range(B)]
    bd_b = [
        np.ascontiguousarray(bond_influence[b].T.astype(np.float16)) for b in range(B)
    ]
    in_maps = []
    for core in range(N_CORES):
        b, g = core // HPC, core % HPC
        s = slice(g * DKG, (g + 1) * DKG)
        bq_g = (bq[s] / 8.0).astype(np.float32)
        bk_g = bk[s].astype(np.float32)
        bqk = np.stack(
            [bq_g[0:128], bq_g[128:256], bk_g[0:128], bk_g[128:256]], axis=1
        )
        in_maps.append(
            {
                "xt": xt_b[b],
                "bd": bd_b[b],
                "wq": np.ascontiguousarray((Wq[:, s] / 8.0).astype(np.float16)),
                "wk": np.ascontiguousarray(Wk[:, s].astype(np.float16)),
                "wv": np.ascontiguousarray(Wv[:, s].astype(np.float16)),
                "bqk": np.ascontiguousarray(bqk),
                "bv": np.ascontiguousarray(bv[s][None, :].astype(np.float16)),
                "wo": np.ascontiguousarray(Wo[s, :].astype(np.float16)),
            }
        )
    return in_maps


def kernel(
    x,
    bond_influence,
    Wq,
    bq,
    Wk,
    bk,
    Wv,
    bv,
    Wo,
    bo,
    _trace=False,
    _trace_out=None,
):
    x = np.asarray(x, dtype=np.float32)
    bond_influence = np.asarray(bond_influence, dtype=np.float32)
    args = [np.asarray(a, dtype=np.float32) for a in (Wq, bq, Wk, bk, Wv, bv, Wo)]
    bo = np.asarray(bo, dtype=np.float32)

    nc = _get_nc()
    in_maps = _host_prep(x, bond_influence, *args)
    kwargs = {}
    if _trace:
        kwargs = dict(trace=True, tmpdir=_trace_out)
    res = run_bass_kernel_spmd(nc, in_maps, list(range(N_CORES)), **kwargs)

    out = np.zeros((B, L, D), dtype=np.float32)
    for b in range(B):
        acc = res.results[4 * b]["y"].astype(np.float32).copy()
        for g in range(1, HPC):
            acc += res.results[4 * b + g]["y"]
        out[b] = acc + bo[None, :]
    if _trace:
        return out, res
    return out

